# revision 6
# baseline (speedup 1.0000x reference)
"""MoE (dense, softmax-routed) Trainium2 kernel.

Problem: x:[4,2048,512] f32, 8 experts with w1:[8,512,2048] b1:[8,2048]
w2:[8,2048,512] b2:[8,512], router wr:[512,8] br:[8].
  rw  = softmax(x @ wr + br)                        -> [4,2048,8]
  h_e = gelu(x @ w1[e] + b1[e])                     (exact gelu)
  out = sum_e rw[..,e] * (h_e @ w2[e] + b2[e])      -> [4,2048,512]
Returns (out, rw).

Sharding: data-parallel over tokens. 8192 tokens are split into 8 slices
of 1024; each NeuronCore runs all 8 experts over its token slice (weights
replicated, no collectives). Host concatenates per-core outputs.

Per-core kernel layout (P=128 partitions):
  - x slice is loaded and transposed once via the PE into xT [d, tok]
    (d chunked by 128), kept resident in SBUF.
  - router: logits[tok,8] = xT.T @ wr (+br via a rank-1 K=1 matmul into
    PSUM), softmax along the free dim.
  - expert loop: weights for expert e stream into SBUF (double-buffered);
    for each 512-token tile, for each 128-wide ff chunk:
      l1_psum[ff,tok] += w1-chunk.T @ xT-chunk   (4 accumulating matmuls)
      h = gelu(l1_psum + b1) via ScalarE (per-partition bias)
      l2_psum[tok,dout] += h-chunk.T @ w2-chunk  (4 token sub-blocks)
    then +b2 as a rank-1 matmul and acc += rw[:,e] * l2_psum on VectorE.
  - Matmuls run as float32r (full-rate PE; fp32 storage) by default.
"""

import os

import numpy as np

import concourse.bass as bass
import concourse.tile as tile
from concourse import bacc, mybir
from concourse import bass_utils
from concourse.masks import make_identity

F32 = mybir.dt.float32

# Problem constants (hardcoded per harness contract)
B, T, DM, FF, NE = 4, 2048, 512, 2048, 8
NCORES = 8
TOK = B * T            # 8192 tokens total
TPC = TOK // NCORES    # 1024 tokens per core
P = 128
DC = DM // P           # 4 d-model chunks
FC = FF // P           # 16 ff chunks
TT = 512               # moving-operand token tile
NT = TPC // TT         # 2 token tiles per core
NB = TPC // P          # 8 token blocks of 128

# matmul compute mode: "f32r" (fp32 storage, float32r PE mode), "bf16", "f32"
MM_MODE = os.environ.get("MOE_MM_MODE", "f32r")


def build_nc(mm_mode: str = MM_MODE, act_fn=None):
    if act_fn is None:
        act_fn = mybir.ActivationFunctionType.Gelu
    if mm_mode == "bf16":
        w_store = mybir.dt.bfloat16   # dram+sbuf storage for w1/w2/wr, h, xT
    elif mm_mode == "f32r":
        # float32r: 4-byte storage, full-rate PE mode. Producers (DMA, DVE
        # copy, ACT) must declare f32r output so values are rounded.
        w_store = mybir.dt.float32r
    elif mm_mode == "f32":
        w_store = F32
    else:
        raise ValueError(mm_mode)

    def mm(ap):
        return ap

    nc = bacc.Bacc("TRN2", target_bir_lowering=False, debug=False,
                   num_devices=NCORES)

    x_d = nc.dram_tensor("x", [TPC, DM], F32, kind="ExternalInput")
    w1_d = nc.dram_tensor("w1", [NE, DM, FF], w_store, kind="ExternalInput")
    b1_d = nc.dram_tensor("b1", [NE, FF], F32, kind="ExternalInput")
    w2_d = nc.dram_tensor("w2", [NE, FF, DM], w_store, kind="ExternalInput")
    b2_d = nc.dram_tensor("b2", [NE, DM], w_store, kind="ExternalInput")
    wr_d = nc.dram_tensor("wr", [DM, NE], w_store, kind="ExternalInput")
    br_d = nc.dram_tensor("br", [NE], w_store, kind="ExternalInput")
    out_d = nc.dram_tensor("out", [TPC, DM], F32, kind="ExternalOutput")
    rw_d = nc.dram_tensor("rw", [TPC, NE], F32, kind="ExternalOutput")

    with tile.TileContext(nc) as tc:
        with (
            tc.tile_pool(name="const", bufs=1) as const_pool,
            tc.tile_pool(name="xT", bufs=1) as xT_pool,
            tc.tile_pool(name="acc", bufs=1) as acc_pool,
            tc.tile_pool(name="rw", bufs=1) as rw_pool,
        ):
            identity = const_pool.tile([P, P], F32, tag="identity")
            make_identity(nc, identity)
            ones1 = const_pool.tile([1, P], w_store, tag="ones1")
            if w_store == mybir.dt.float32r:
                ones1_f = const_pool.tile([1, P], F32, tag="ones1_f")
                nc.vector.memset(ones1_f[:], 1.0)
                nc.vector.tensor_copy(ones1[:], ones1_f[:])
            else:
                nc.vector.memset(ones1[:], 1.0)
            wr_sb = const_pool.tile([P, DC, NE], w_store, tag="wr_sb")
            nc.sync.dma_start(wr_sb[:], wr_d.ap().rearrange("(c p) e -> p c e", p=P))
            br_sb = const_pool.tile([1, NE], w_store, tag="br_sb")
            nc.sync.dma_start(br_sb[:], br_d.ap().rearrange("(o e) -> o e", o=1))

            xT_sb = xT_pool.tile([P, DC, TPC], w_store, tag="xT_sb")
            acc = acc_pool.tile([P, NB, DM], F32, tag="acc")
            rw_sb = rw_pool.tile([P, NB, NE], F32, tag="rw_sb")

            # ---- Phase A: load x, transpose to xT[d, tok] ----
            with (
                tc.tile_pool(name="xin", bufs=2) as xin_pool,
                tc.tile_pool(name="ps_a", bufs=2, space="PSUM") as ps_a,
            ):
                for blk in range(NB):
                    xt = xin_pool.tile([P, DM], F32, tag="xt", bufs=2)
                    nc.sync.dma_start(xt[:], x_d.ap()[blk * P:(blk + 1) * P, :])
                    for dc in range(DC):
                        tp = ps_a.tile([P, P], F32, tag="tp", bufs=4)
                        nc.tensor.transpose(tp[:], xt[:, dc * P:(dc + 1) * P],
                                            identity[:])
                        nc.vector.tensor_copy(
                            xT_sb[:, dc, blk * P:(blk + 1) * P], tp[:])

                # ---- Phase B: router (uses xT) ----
                for blk in range(NB):
                    lg = ps_a.tile([P, NE], F32, tag="lg", bufs=2)
                    for dc in range(DC):
                        nc.tensor.matmul(
                            lg[:],
                            lhsT=mm(xT_sb[:, dc, blk * P:(blk + 1) * P]),
                            rhs=mm(wr_sb[:, dc, :]),
                            start=(dc == 0), stop=False)
                    # + br (rank-1: ones[1,P].T @ br[1,NE])
                    nc.tensor.matmul(lg[:], lhsT=mm(ones1[:]), rhs=mm(br_sb[:]),
                                     start=False, stop=True)
                    ex = rw_pool.tile([P, NE], F32, tag="ex", bufs=2)
                    nc.scalar.activation(ex[:], lg[:],
                                         mybir.ActivationFunctionType.Exp)
                    sm = rw_pool.tile([P, 1], F32, tag="sm", bufs=2)
                    nc.vector.reduce_sum(sm[:], ex[:], axis=mybir.AxisListType.X)
                    rec = rw_pool.tile([P, 1], F32, tag="rec", bufs=2)
                    nc.vector.reciprocal(rec[:], sm[:])
                    nc.vector.tensor_scalar_mul(rw_sb[:, blk, :], ex[:], rec[:])
                    nc.sync.dma_start(rw_d.ap()[blk * P:(blk + 1) * P, :],
                                      rw_sb[:, blk, :])

            # ---- Phase C: expert loop ----
            with (
                tc.tile_pool(name="wpool", bufs=2) as wpool,
                tc.tile_pool(name="hpool", bufs=3) as hpool,
                tc.tile_pool(name="epil", bufs=2) as epil,
                tc.tile_pool(name="ps_l1", bufs=2, space="PSUM") as ps_l1,
                tc.tile_pool(name="ps_l2", bufs=1, space="PSUM") as ps_l2,
            ):
                for e in range(NE):
                    w1s = wpool.tile([P, DC, FF], w_store, tag="w1s", bufs=2)
                    nc.sync.dma_start(
                        w1s[:], w1_d.ap()[e].rearrange("(c p) f -> p c f", p=P))
                    w2s = wpool.tile([P, FC, DM], w_store, tag="w2s", bufs=2)
                    nc.sync.dma_start(
                        w2s[:], w2_d.ap()[e].rearrange("(c p) d -> p c d", p=P))
                    b1s = wpool.tile([P, FC], F32, tag="b1s", bufs=2)
                    nc.sync.dma_start(
                        b1s[:], b1_d.ap()[e].rearrange("(c p) -> p c", p=P))
                    b2s = wpool.tile([1, DM], w_store, tag="b2s", bufs=2)
                    nc.sync.dma_start(b2s[:], b2_d.ap()[e:e + 1, :])

                    for tt in range(NT):
                        l2 = ps_l2.tile([P, 4, DM], F32, tag="l2", bufs=1)
                        for fc in range(FC):
                            l1 = ps_l1.tile([P, TT], F32, tag="l1", bufs=2)
                            for dc in range(DC):
                                nc.tensor.matmul(
                                    l1[:],
                                    lhsT=mm(w1s[:, dc, fc * P:(fc + 1) * P]),
                                    rhs=mm(xT_sb[:, dc, tt * TT:(tt + 1) * TT]),
                                    start=(dc == 0), stop=(dc == DC - 1))
                            h = hpool.tile([P, TT], w_store, tag="h", bufs=3)
                            nc.scalar.activation(
                                h[:], l1[:], act_fn,
                                bias=b1s[:, fc:fc + 1])
                            for ts in range(4):
                                nc.tensor.matmul(
                                    l2[:, ts, :],
                                    lhsT=mm(h[:, ts * P:(ts + 1) * P]),
                                    rhs=mm(w2s[:, fc, :]),
                                    start=(fc == 0), stop=False)
                        for ts in range(4):
                            # + b2 (rank-1), closes the accumulation group
                            nc.tensor.matmul(l2[:, ts, :], lhsT=mm(ones1[:]),
                                             rhs=mm(b2s[:]),
                                             start=False, stop=True)
                            blk = tt * 4 + ts
                            rwcol = rw_sb[:, blk, e:e + 1]
                            if e == 0:
                                nc.vector.tensor_scalar_mul(
                                    acc[:, blk, :], l2[:, ts, :], rwcol)
                            else:
                                tmp = epil.tile([P, DM], F32, tag="tmp", bufs=2)
                                nc.vector.tensor_scalar_mul(
                                    tmp[:], l2[:, ts, :], rwcol)
                                nc.vector.tensor_add(
                                    acc[:, blk, :], acc[:, blk, :], tmp[:])

            # ---- Phase D: store outputs ----
            for blk in range(NB):
                nc.sync.dma_start(out_d.ap()[blk * P:(blk + 1) * P, :],
                                  acc[:, blk, :])

    nc.compile()
    return nc


_NC_CACHE = {}


def _get_nc(mm_mode: str = MM_MODE):
    if mm_mode not in _NC_CACHE:
        _NC_CACHE[mm_mode] = build_nc(mm_mode)
    return _NC_CACHE[mm_mode]


def make_in_maps(inputs: dict, mm_mode: str = MM_MODE):
    import ml_dtypes

    x = np.ascontiguousarray(np.asarray(inputs["x"], np.float32))
    xf = x.reshape(TOK, DM)
    if mm_mode == "bf16":
        wdt = ml_dtypes.bfloat16
    else:
        wdt = np.float32
    w1 = np.asarray(inputs["w1"], np.float32).astype(wdt)
    w2 = np.asarray(inputs["w2"], np.float32).astype(wdt)
    wr = np.asarray(inputs["wr"], np.float32).astype(wdt)
    br = np.asarray(inputs["br"], np.float32).astype(wdt)
    b2 = np.asarray(inputs["b2"], np.float32).astype(wdt)
    b1 = np.asarray(inputs["b1"], np.float32)
    in_maps = []
    for c in range(NCORES):
        in_maps.append({
            "x": np.ascontiguousarray(xf[c * TPC:(c + 1) * TPC]),
            "w1": w1, "b1": b1, "w2": w2, "b2": b2, "wr": wr, "br": br,
        })
    return in_maps


def kernel(**inputs):
    nc = _get_nc()
    in_maps = make_in_maps(inputs)
    res = bass_utils.run_bass_kernel_spmd(nc, in_maps,
                                          core_ids=list(range(NCORES)))
    out = np.concatenate([res.results[c]["out"] for c in range(NCORES)],
                         axis=0).reshape(B, T, DM)
    rw = np.concatenate([res.results[c]["rw"] for c in range(NCORES)],
                        axis=0).reshape(B, T, NE)
    return out, rw
